# revision 10
# baseline (speedup 1.0000x reference)
"""Trainium2 Bass kernel for nn_Embedded_GCN (gnn_message_passing).

Reference math (B=32, N=4096, C=32, O=64, D=16, K=3):
  A  = softmax(relu(E @ E.T), axis=1)              # [N, N] adaptive adjacency
  T0 = I, T1 = A, T2 = 2A@A - I                    # Chebyshev
  x_g[k]   = T_k @ x_b  for each batch             # [B, K, N, C]
  W[n]     = sum_d E[n,d] * Wp[d]                  # per-node weights [K,C,O]
  out[b,n] = sum_{k,i} x_g[b,n,k,i] W[n,k,i,:] + E[n]@bias_pool

Structure (v2 — latency-hiding rewrite):
  * T2 never materialized: z1 = A@x, z2 = 2*A@z1 - x.
  * softmax(relu(s)) = max(1, exp(s)) / rowsum; scores f32r, hops bf16.
  * PT[m, n-local] computed transposed so hops need no transpose.
  * hop1 runs as two column sweeps (cb 0:512 then 512:1024): the first
    AllGather (z1 cols h0, all 8 cores) launches at hop1's midpoint and
    overlaps sweep B + weight generation; AG1 overlaps hop2's first rows.
    Same total x traffic (each half-column block read once).
  * Weight slab generated with 4x row-tiled matmuls (tile_position rows
    0/32/64/96 via replicated operands) so LDWEIGHTS of tile g+1 overlaps
    the matmul of tile g (contraction is only D=16).
  * hop2 runs as two node sweeps (nt{0,1} then nt{2,3}); chunk 0/1 xg
    assembly + grouped GEMMs overlap sweep B (ag_out re-read once more).
  * xg k=0 rows + the ones row come pre-permuted from the host; k=1/k=2
    rows go through a DRAM bounce (scatter-write, contiguous read).
  * Output written as [NL, B, O] (8KB contiguous per node); host
    transposes back to [B, NL, O].
"""

import os

import numpy as np
import ml_dtypes

import concourse.bass as bass
import concourse.mybir as mybir
import concourse.tile as tile
from concourse import bacc
from concourse.bass_utils import run_bass_kernel_spmd

F32 = mybir.dt.float32
F32R = mybir.dt.float32r
BF16 = mybir.dt.bfloat16
AF = mybir.ActivationFunctionType

B, N, C, O, D, CHEB_K = 32, 4096, 32, 64, 16, 3
NC_CORES = 8
NL = N // NC_CORES          # 512 nodes per core
BC = B * C                  # 1024
H = BC // 2                 # 512 column half
MT = N // 128               # 32 contraction tiles
NT = NL // 128              # 4 local node tiles
KC = CHEB_K * C             # 96

LAST_RESULTS = {}


def _register_ntff_hook():
    """Inject antenv.axon_hooks (absent from the container's antenv stub) and
    register the ctypes NTFF-profile hook so trace=True works under axon."""
    import sys
    import types

    try:
        import antenv

        if "antenv.axon_hooks" not in sys.modules:
            mod = types.ModuleType("antenv.axon_hooks")
            mod._hook = None

            def set_axon_ntff_profile_hook(h):
                mod._hook = h

            def get_axon_ntff_profile_hook():
                return mod._hook

            mod.set_axon_ntff_profile_hook = set_axon_ntff_profile_hook
            mod.get_axon_ntff_profile_hook = get_axon_ntff_profile_hook
            sys.modules["antenv.axon_hooks"] = mod
            antenv.axon_hooks = mod

        hooks = sys.modules["antenv.axon_hooks"]
        if hooks.get_axon_ntff_profile_hook() is None:
            from trn_agent_boot.trn_boot import _ntff_profile_via_ctypes

            hook = _ntff_profile_via_ctypes("/opt/axon/libaxon_pjrt.so")
            if hook is not None:
                hooks.set_axon_ntff_profile_hook(hook)
        return True
    except Exception:
        return False


def _build(nc: bacc.Bacc):
    # ---- I/O -------------------------------------------------------------
    et = nc.dram_tensor("et", [D, N], F32, kind="ExternalInput")          # E^T
    et_loc = nc.dram_tensor("et_loc", [D, NL], F32, kind="ExternalInput")
    et_loc_bf = nc.dram_tensor("et_loc_bf", [D, NL], BF16, kind="ExternalInput")
    xt_h0 = nc.dram_tensor("xt_h0", [N, H], BF16, kind="ExternalInput")   # x[m, cb 0:512]
    xt_h1 = nc.dram_tensor("xt_h1", [N, H], BF16, kind="ExternalInput")   # x[m, cb 512:1024]
    xt_loc = nc.dram_tensor("xt_loc", [NL, BC], F32, kind="ExternalInput")
    xg0 = nc.dram_tensor("xg0", [C + 1, NL * B], BF16, kind="ExternalInput")
    wpo4 = nc.dram_tensor("wpo4", [3, D, 22 * KC], BF16, kind="ExternalInput")
    bias_flat = nc.dram_tensor("bias_flat", [1, NL * O], BF16, kind="ExternalInput")
    out_loc = nc.dram_tensor("out_loc", [NL, B, O], F32, kind="ExternalOutput")

    with tile.TileContext(nc) as tc:
        with tc.tile_pool(name="dram", bufs=1, space="DRAM") as dram, \
             tc.tile_pool(name="persist", bufs=1) as persist:

            ag_ins = [dram.tile([NL, H], BF16, tag=f"ag_in{q}", name=f"ag_in{q}")
                      for q in range(2)]
            ag_outs = [dram.tile([N, H], BF16, tag=f"ag_out{q}",
                                 name=f"ag_out{q}", addr_space="Shared")
                       for q in range(2)]
            scr1 = dram.tile([C, NL, B], BF16, tag="scr1")   # z1 as [c, n, b]
            scr2 = dram.tile([C, NL, B], BF16, tag="scr2")   # z2 as [c, n, b]

            # ---- small persistent SBUF ------------------------------------
            etl_sb = persist.tile([D, NL], F32R, tag="etl")
            r1 = persist.tile([128, NT], F32, tag="r1")          # 1/Z per node col nt
            r2 = persist.tile([128, NT], F32, tag="r2")          # 2/Z
            etl_rep = persist.tile([96, NL], BF16, tag="etlrep")
            wpo_rep = persist.tile([96, 22 * KC], BF16, tag="wporep")
            ones_f = persist.tile([128, 2], F32, tag="onesf")
            xloc_sb = persist.tile([128, NT * BC], F32, tag="xloc")

            nc.sync.dma_start(etl_sb[:], et_loc[:, :].bitcast(F32R))
            for g in range(3):
                nc.gpsimd.dma_start(etl_rep[32 * g:32 * g + D, :], et_loc_bf[:, :])
                nc.scalar.dma_start(wpo_rep[32 * g:32 * g + D, :], wpo4[g, :, :])
            nc.vector.memset(ones_f[:], 1.0)
            nc.gpsimd.dma_start(
                xloc_sb[:].rearrange("p (t f) -> p t f", f=BC),
                xt_loc[:, :].rearrange("(t p) f -> p t f", p=128),
            )

            with tc.tile_pool(name="wtp", bufs=1) as wtp, \
                 tc.tile_pool(name="tstream", bufs=3) as tstream:
                # weight slab, n-major: [kc|bias, (n_hi, o, n_lo=8)]
                wt_bf = wtp.tile([KC + 1, NL * O], BF16, tag="wt")
                nc.gpsimd.dma_start(wt_bf[KC:KC + 1, :], bias_flat[:, :])
                wt_i8 = wt_bf[0:KC, :].rearrange("p (nh o nl) -> p nh o nl", o=O, nl=8)
                wt_g = wt_bf[:].rearrange("p (nh o nl) -> p nh nl o", o=O, nl=8)

                xgp_cm = tc.tile_pool(name="xg", bufs=1)
                xgp = xgp_cm.__enter__()
                xgs = []
                for ch in range(NT):
                    xg_t = xgp.tile([KC + 1, 128 * B], BF16, tag=f"xg{ch}", name=f"xg{ch}")
                    # k=0 rows (x, pre-permuted on host) + the ones row
                    nc.sync.dma_start(xg_t[0:C, :], xg0[0:C, ch * 128 * B:(ch + 1) * 128 * B])
                    nc.scalar.dma_start(xg_t[KC:KC + 1, :], xg0[C:C + 1, ch * 128 * B:(ch + 1) * 128 * B])
                    xgs.append(xg_t)

                with tc.tile_pool(name="ptp", bufs=1) as ptp, \
                     tc.tile_pool(name="stream", bufs=3) as stream:
                    pt = ptp.tile([128, MT * NL], BF16, tag="pt")  # PT[m%128, mt*NL+n]

                    # ---- transposed exp-scores + row sums -----------------
                    with tc.tile_pool(name="etp", bufs=2) as etp, \
                         tc.tile_pool(name="ps_sc", bufs=4, space="PSUM") as ps_sc:
                        accs = [ptp.tile([128, NL], F32, tag=f"accs{i}", name=f"accs{i}")
                                for i in range(2)]
                        et_c = None
                        for mt in range(MT):
                            if mt % 8 == 0:
                                et_c = etp.tile([D, 1024], F32R, tag="etc")
                                nc.scalar.dma_start(
                                    et_c[:],
                                    et[:, mt * 128:(mt + 8) * 128].bitcast(F32R))
                            s_ps = ps_sc.tile([128, NL], F32, tag="s")
                            nc.tensor.matmul(
                                s_ps[:],
                                et_c[:, (mt % 8) * 128:(mt % 8 + 1) * 128],
                                etl_sb[:],
                                start=True, stop=True,
                            )
                            pslice = pt[:, mt * NL:(mt + 1) * NL]
                            nc.scalar.activation(pslice, s_ps[:], AF.Exp)
                            nc.vector.tensor_scalar_max(pslice, pslice, 1.0)
                            a = accs[mt % 2]
                            if mt < 2:
                                nc.vector.tensor_copy(a[:], pslice)
                            else:
                                nc.vector.tensor_tensor(
                                    a[:], a[:], pslice, mybir.AluOpType.add)
                        nc.vector.tensor_tensor(
                            accs[0][:], accs[0][:], accs[1][:], mybir.AluOpType.add)
                        acc_fin = accs[0]

                    # ---- hop1: two column sweeps, AG per sweep ------------
                    st1s = {}
                    with tc.tile_pool(name="ps_h1", bufs=1, space="PSUM") as ps_h1:
                        xt_hs = [xt_h0, xt_h1]
                        for h in range(2):
                            acc = [ps_h1.tile([128, H], F32, tag=f"acc{nt_i}",
                                              name=f"acc{h}_{nt_i}")
                                   for nt_i in range(NT)]
                            for k in range(MT):
                                rt = stream.tile([128, H], BF16, tag="rhs", bufs=3)
                                nc.sync.dma_start(rt[:], xt_hs[h][k * 128:(k + 1) * 128, :])
                                for nt_i in range(NT):
                                    lhs = pt[:, k * NL + nt_i * 128: k * NL + (nt_i + 1) * 128]
                                    nc.tensor.matmul(
                                        acc[nt_i][:], lhs, rt[:],
                                        start=(k == 0), stop=(k == MT - 1),
                                    )
                            if h == 0:
                                # row sums -> 1/Z, 2/Z (overlaps sweep B)
                                with tc.tile_pool(name="ps_zs", bufs=1, space="PSUM") as ps_zs:
                                    zs = ps_zs.tile([128, 2 * NT], F32, tag="zs")
                                    for nt_i in range(NT):
                                        nc.tensor.matmul(
                                            zs[:, 2 * nt_i:2 * nt_i + 2],
                                            acc_fin[:, nt_i * 128:(nt_i + 1) * 128],
                                            ones_f[:],
                                            start=True, stop=True,
                                        )
                                    for nt_i in range(NT):
                                        nc.vector.reciprocal(
                                            r1[:, nt_i:nt_i + 1], zs[:, 2 * nt_i:2 * nt_i + 1])
                                        nc.vector.tensor_scalar_mul(
                                            r2[:, nt_i:nt_i + 1], r1[:, nt_i:nt_i + 1], 2.0)
                            # epilogue: scale by 1/Z, stage AG input, launch AG
                            for nt_i in range(NT):
                                st_bf = stream.tile([128, H], BF16, tag=f"z1_{h}_{nt_i}",
                                                    name=f"z1_{h}_{nt_i}", bufs=1)
                                nc.vector.tensor_scalar(
                                    st_bf[:], acc[nt_i][:], r1[:, nt_i:nt_i + 1], None,
                                    op0=mybir.AluOpType.mult,
                                )
                                nc.gpsimd.dma_start(
                                    ag_ins[h][nt_i * 128:(nt_i + 1) * 128, :], st_bf[:])
                                st1s[(h, nt_i)] = st_bf
                            nc.gpsimd.collective_compute(
                                "AllGather",
                                mybir.AluOpType.bypass,
                                ins=[ag_ins[h].opt()],
                                outs=[ag_outs[h].opt()],
                                replica_groups=[list(range(NC_CORES))],
                            )
                            # z1 -> scr1 (c-major scatter; feeds xg k=1 rows)
                            for nt_i in range(NT):
                                nc.scalar.dma_start(
                                    scr1[h * 16:(h + 1) * 16, nt_i * 128:(nt_i + 1) * 128, :]
                                    .transpose((1, 0, 2)),
                                    st1s[(h, nt_i)][:].rearrange("p (c b) -> p c b", b=B),
                                )
                            for ch in range(NT):
                                nc.gpsimd.dma_start(
                                    xgs[ch][C + 16 * h:C + 16 * (h + 1), :]
                                    .rearrange("c (n b) -> c n b", b=B),
                                    scr1[h * 16:(h + 1) * 16, ch * 128:(ch + 1) * 128, :],
                                )

                    # ---- weight-slab generation (4x row-tiled) ------------
                    with tc.tile_pool(name="ps_wt", bufs=4, space="PSUM") as ps_wt:
                        for o in range(O):
                            g, j = o % 3, o // 3
                            w_ps = ps_wt.tile([KC, H], F32, tag="wps")
                            nc.tensor.matmul(
                                w_ps[:],
                                wpo_rep[32 * g:32 * g + D, j * KC:(j + 1) * KC],
                                etl_rep[32 * g:32 * g + D, :],
                                start=True, stop=True,
                            )
                            src_v = w_ps[:].rearrange("p (nh nl) -> p nh nl", nl=8)
                            if o % 2 == 0:
                                nc.vector.tensor_copy(wt_i8[:, :, o, :], src_v)
                            else:
                                nc.scalar.activation(wt_i8[:, :, o, :], src_v, AF.Copy)

                    # ---- hop2: two node sweeps + pipelined grouped GEMM ---
                    def hop2_sweep(nts, ps_h2):
                        acc2 = {}
                        for h in range(2):
                            for nt_i in nts:
                                acc2[(nt_i, h)] = ps_h2.tile(
                                    [128, H], F32, tag=f"a2_{nt_i % 2}_{h}",
                                    name=f"a2_{nt_i}_{h}")
                            for k in range(MT):
                                rt = stream.tile([128, H], BF16, tag="rhs", bufs=3)
                                nc.sync.dma_start(
                                    rt[:], ag_outs[h][k * 128:(k + 1) * 128, :])
                                for nt_i in nts:
                                    lhs = pt[:, k * NL + nt_i * 128: k * NL + (nt_i + 1) * 128]
                                    nc.tensor.matmul(
                                        acc2[(nt_i, h)][:], lhs, rt[:],
                                        start=(k == 0), stop=(k == MT - 1),
                                    )
                            # epilogue for this column half (overlaps h=1 matmuls)
                            for nt_i in nts:
                                st = stream.tile([128, H], F32, tag="zst", bufs=2)
                                nc.scalar.activation(
                                    st[:], acc2[(nt_i, h)][:], AF.Copy,
                                    scale=r2[:, nt_i:nt_i + 1],
                                )
                                st_bf = stream.tile([128, H], BF16, tag="zstb", bufs=2)
                                nc.vector.tensor_tensor(
                                    st_bf[:], st[:],
                                    xloc_sb[:, nt_i * BC + h * H: nt_i * BC + (h + 1) * H],
                                    mybir.AluOpType.subtract,
                                )
                                nc.gpsimd.dma_start(
                                    scr2[h * 16:(h + 1) * 16, nt_i * 128:(nt_i + 1) * 128, :]
                                    .transpose((1, 0, 2)),
                                    st_bf[:].rearrange("p (c b) -> p c b", b=B),
                                )
                        for ch in nts:
                            for h in range(2):
                                nc.scalar.dma_start(
                                    xgs[ch][2 * C + 16 * h:2 * C + 16 * (h + 1), :]
                                    .rearrange("c (n b) -> c n b", b=B),
                                    scr2[16 * h:16 * (h + 1), ch * 128:(ch + 1) * 128, :],
                                )

                    def gemm_chunks(chs, ps_g):
                        for ch in chs:
                            n0 = ch * 128
                            xg_b = xgs[ch]
                            for q16 in range(4):  # 32 nodes per psum tile
                                g_ps = ps_g.tile([128, 512], F32, tag="gps")
                                for j in range(8):
                                    for g in range(4):
                                        nl_i = q16 * 32 + j * 4 + g
                                        n_gl = n0 + nl_i
                                        nc.tensor.matmul(
                                            g_ps[32 * g:32 * (g + 1), j * O:(j + 1) * O],
                                            xg_b[:, nl_i * B:(nl_i + 1) * B],
                                            wt_g[:, n_gl // 8, n_gl % 8, :],
                                            start=True, stop=True,
                                            tile_position=(0, 32 * g),
                                        )
                                st = tstream.tile([128, 512], F32, tag="gst", bufs=2)
                                nc.vector.tensor_copy(st[:], g_ps[:])
                                dst4 = out_loc[n0 + q16 * 32: n0 + (q16 + 1) * 32, :, :] \
                                    .rearrange("(j g) b o -> g b j o", g=4)
                                for g in range(4):
                                    eng = (nc.sync, nc.gpsimd, nc.scalar, nc.sync)[g]
                                    eng.dma_start(
                                        dst4[g],
                                        st[32 * g:32 * (g + 1), :].rearrange(
                                            "b (j o) -> b j o", o=O),
                                    )

                    with tc.tile_pool(name="ps_h2", bufs=1, space="PSUM") as ps_h2, \
                         tc.tile_pool(name="ps_g", bufs=2, space="PSUM") as ps_g:
                        hop2_sweep([0, 1], ps_h2)
                        hop2_sweep([2, 3], ps_h2)
                        gemm_chunks([0, 1], ps_g)
                        gemm_chunks([2, 3], ps_g)
                xgp_cm.__exit__(None, None, None)
    return out_loc


_COMPILED = None


def _get_compiled():
    global _COMPILED
    if _COMPILED is None:
        nc = bacc.Bacc(
            "TRN2",
            target_bir_lowering=False,
            debug=False,
            num_devices=NC_CORES,
        )
        _build(nc)
        nc.compile()
        _COMPILED = nc
    return _COMPILED


def kernel(x, node_embeddings, laplacian_mx, weights_pool, bias_pool):
    x = np.asarray(x, dtype=np.float32)
    e = np.asarray(node_embeddings, dtype=np.float32)
    wp = np.asarray(weights_pool, dtype=np.float32)
    bp = np.asarray(bias_pool, dtype=np.float32)

    et = np.ascontiguousarray(e.T)                                    # [D, N]
    xt_h = np.ascontiguousarray(x.transpose(1, 2, 0).reshape(N, BC))  # [m, c*32+b]
    xt_b = xt_h.astype(ml_dtypes.bfloat16)
    xt_b0 = np.ascontiguousarray(xt_b[:, :H])
    xt_b1 = np.ascontiguousarray(xt_b[:, H:])
    bias_h = (e @ bp).astype(np.float32)                              # [N, O]

    # wpo4[g] = [d, (j, kc)] where o = j*3 + g (padded to 22 j-slots)
    wpo = wp.reshape(D, KC, O)                                        # [d, kc, o]
    wpo4 = np.zeros((3, D, 22, KC), dtype=np.float32)
    for g in range(3):
        sel = wpo[:, :, g::3]                                         # [d, kc, nj]
        wpo4[g, :, :sel.shape[2], :] = sel.transpose(0, 2, 1)
    wpo4 = np.ascontiguousarray(wpo4.reshape(3, D, 22 * KC)).astype(ml_dtypes.bfloat16)

    et_bf = et.astype(ml_dtypes.bfloat16)
    ones_row = np.ones((1, NL * B), dtype=ml_dtypes.bfloat16)

    in_maps = []
    for i in range(NC_CORES):
        sl = slice(i * NL, (i + 1) * NL)
        # xg k=0 slab: [c, n, b] for local nodes + ones row
        xg0_i = np.concatenate([
            np.ascontiguousarray(x[:, sl, :].transpose(2, 1, 0)).reshape(C, NL * B)
            .astype(ml_dtypes.bfloat16),
            ones_row,
        ], axis=0)
        in_maps.append({
            "et": et,
            "et_loc": np.ascontiguousarray(et[:, sl]),
            "et_loc_bf": np.ascontiguousarray(et_bf[:, sl]),
            "xt_h0": xt_b0,
            "xt_h1": xt_b1,
            "xt_loc": np.ascontiguousarray(xt_h[sl]),
            "xg0": xg0_i,
            "wpo4": wpo4,
            "bias_flat": np.ascontiguousarray(
                bias_h[sl].reshape(64, 8, O).transpose(0, 2, 1).reshape(1, NL * O)
                .astype(ml_dtypes.bfloat16)
            ),
        })

    nc = _get_compiled()
    trace = bool(int(os.environ.get("KBENCH_TRACE", "0")))
    if trace:
        trace = _register_ntff_hook()
    res = run_bass_kernel_spmd(
        nc,
        in_maps,
        core_ids=list(range(NC_CORES)),
        trace=trace,
    )
    LAST_RESULTS["exec_time_ns"] = res.exec_time_ns
    LAST_RESULTS["trace"] = res.instructions_and_trace
    LAST_RESULTS["mean_exec_time_ns"] = res.mean_exec_time_ns

    out = np.empty((B, N, O), dtype=np.float32)
    for i in range(NC_CORES):
        out[:, i * NL:(i + 1) * NL, :] = res.results[i]["out_loc"].transpose(1, 0, 2)
    return out


# revision 14
# speedup vs baseline: 1.3686x; 1.3686x over previous
"""Trainium2 Bass kernel for nn_Embedded_GCN (gnn_message_passing).

Reference math (B=32, N=4096, C=32, O=64, D=16, K=3):
  A  = softmax(relu(E @ E.T), axis=1)              # [N, N] adaptive adjacency
  T0 = I, T1 = A, T2 = 2A@A - I                    # Chebyshev
  x_g[k]   = T_k @ x_b  for each batch             # [B, K, N, C]
  W[n]     = sum_d E[n,d] * Wp[d]                  # per-node weights [K,C,O]
  out[b,n] = sum_{k,i} x_g[b,n,k,i] W[n,k,i,:] + E[n]@bias_pool

Structure (v2 — latency-hiding rewrite):
  * T2 never materialized: z1 = A@x, z2 = 2*A@z1 - x.
  * softmax(relu(s)) = max(1, exp(s)) / rowsum; scores f32r, hops bf16.
  * PT[m, n-local] computed transposed so hops need no transpose.
  * hop1 runs as two column sweeps (cb 0:512 then 512:1024): the first
    AllGather (z1 cols h0, all 8 cores) launches at hop1's midpoint and
    overlaps sweep B + weight generation; AG1 overlaps hop2's first rows.
    Same total x traffic (each half-column block read once).
  * Weight slab generated with 4x row-tiled matmuls (tile_position rows
    0/32/64/96 via replicated operands) so LDWEIGHTS of tile g+1 overlaps
    the matmul of tile g (contraction is only D=16).
  * hop2 runs as two node sweeps (nt{0,1} then nt{2,3}); chunk 0/1 xg
    assembly + grouped GEMMs overlap sweep B (ag_out re-read once more).
  * xg k=0 rows + the ones row come pre-permuted from the host; k=1/k=2
    rows go through a DRAM bounce (scatter-write, contiguous read).
  * Output written as [NL, B, O] (8KB contiguous per node); host
    transposes back to [B, NL, O].
"""

import os

import numpy as np
import ml_dtypes

import concourse.bass as bass
import concourse.mybir as mybir
import concourse.tile as tile
from concourse import bacc
from concourse.bass_utils import run_bass_kernel_spmd

F32 = mybir.dt.float32
F32R = mybir.dt.float32r
BF16 = mybir.dt.bfloat16
AF = mybir.ActivationFunctionType

B, N, C, O, D, CHEB_K = 32, 4096, 32, 64, 16, 3
NC_CORES = 8
NL = N // NC_CORES          # 512 nodes per core
BC = B * C                  # 1024
H = BC // 2                 # 512 column half
MT = N // 128               # 32 contraction tiles
NT = NL // 128              # 4 local node tiles
KC = CHEB_K * C             # 96

LAST_RESULTS = {}


def _register_ntff_hook():
    """Inject antenv.axon_hooks (absent from the container's antenv stub) and
    register the ctypes NTFF-profile hook so trace=True works under axon."""
    import sys
    import types

    try:
        import antenv

        if "antenv.axon_hooks" not in sys.modules:
            mod = types.ModuleType("antenv.axon_hooks")
            mod._hook = None

            def set_axon_ntff_profile_hook(h):
                mod._hook = h

            def get_axon_ntff_profile_hook():
                return mod._hook

            mod.set_axon_ntff_profile_hook = set_axon_ntff_profile_hook
            mod.get_axon_ntff_profile_hook = get_axon_ntff_profile_hook
            sys.modules["antenv.axon_hooks"] = mod
            antenv.axon_hooks = mod

        hooks = sys.modules["antenv.axon_hooks"]
        if hooks.get_axon_ntff_profile_hook() is None:
            from trn_agent_boot.trn_boot import _ntff_profile_via_ctypes

            hook = _ntff_profile_via_ctypes("/opt/axon/libaxon_pjrt.so")
            if hook is not None:
                hooks.set_axon_ntff_profile_hook(hook)
        return True
    except Exception:
        return False


def _build(nc: bacc.Bacc):
    # ---- I/O -------------------------------------------------------------
    et = nc.dram_tensor("et", [D, N], F32, kind="ExternalInput")          # E^T
    et_loc = nc.dram_tensor("et_loc", [D, NL], F32, kind="ExternalInput")
    et_loc_bf = nc.dram_tensor("et_loc_bf", [D, NL], BF16, kind="ExternalInput")
    xt_h0 = nc.dram_tensor("xt_h0", [N, H], BF16, kind="ExternalInput")   # x[m, cb 0:512]
    xt_h1 = nc.dram_tensor("xt_h1", [N, H], BF16, kind="ExternalInput")   # x[m, cb 512:1024]
    xt_loc = nc.dram_tensor("xt_loc", [NL, BC], F32, kind="ExternalInput")
    xg0 = nc.dram_tensor("xg0", [C + 1, NL * B], BF16, kind="ExternalInput")
    wpo4 = nc.dram_tensor("wpo4", [3, D, 22 * KC], BF16, kind="ExternalInput")
    bias_flat = nc.dram_tensor("bias_flat", [1, NL * O], BF16, kind="ExternalInput")
    out_loc = nc.dram_tensor("out_loc", [NL, B, O], F32, kind="ExternalOutput")

    with tile.TileContext(nc) as tc:
        with tc.tile_pool(name="dram", bufs=1, space="DRAM") as dram, \
             tc.tile_pool(name="persist", bufs=1) as persist:

            ag_ins = [dram.tile([NL, H], BF16, tag=f"ag_in{q}", name=f"ag_in{q}")
                      for q in range(2)]
            ag_outs = [dram.tile([N, H], BF16, tag=f"ag_out{q}",
                                 name=f"ag_out{q}", addr_space="Shared")
                       for q in range(2)]
            scr1 = dram.tile([C, NL, B], BF16, tag="scr1")   # z1 as [c, n, b]
            scr2 = dram.tile([C, NL, B], BF16, tag="scr2")   # z2 as [c, n, b]

            # ---- small persistent SBUF ------------------------------------
            etl_sb = persist.tile([D, NL], F32R, tag="etl")
            r1 = persist.tile([128, NT], F32, tag="r1")          # 1/Z per node col nt
            r2 = persist.tile([128, NT], F32, tag="r2")          # 2/Z
            etl_rep = persist.tile([96, NL], BF16, tag="etlrep")
            wpo_rep = persist.tile([96, 22 * KC], BF16, tag="wporep")
            ones_f = persist.tile([128, 2], F32, tag="onesf")
            xloc_sb = persist.tile([128, NT * BC], F32, tag="xloc")

            nc.sync.dma_start(etl_sb[:], et_loc[:, :].bitcast(F32R))
            for g in range(3):
                nc.gpsimd.dma_start(etl_rep[32 * g:32 * g + D, :], et_loc_bf[:, :])
                nc.gpsimd.dma_start(wpo_rep[32 * g:32 * g + D, :], wpo4[g, :, :])
            nc.vector.memset(ones_f[:], 1.0)
            nc.gpsimd.dma_start(
                xloc_sb[:].rearrange("p (t f) -> p t f", f=BC),
                xt_loc[:, :].rearrange("(t p) f -> p t f", p=128),
            )

            with tc.tile_pool(name="wtp", bufs=1) as wtp, \
                 tc.tile_pool(name="tstream", bufs=3) as tstream:
                # weight slab, n-major: [kc|bias, (n_hi, o, n_lo=8)]
                wt_bf = wtp.tile([KC + 1, NL * O], BF16, tag="wt")
                nc.gpsimd.dma_start(wt_bf[KC:KC + 1, :], bias_flat[:, :])
                wt_i8 = wt_bf[0:KC, :].rearrange("p (nh o nl) -> p nh o nl", o=O, nl=8)
                wt_g = wt_bf[:].rearrange("p (nh o nl) -> p nh nl o", o=O, nl=8)

                xgp_cm = tc.tile_pool(name="xg", bufs=1)
                xgp = xgp_cm.__enter__()
                xgs = []
                for ch in range(NT):
                    xg_t = xgp.tile([KC + 1, 128 * B], BF16, tag=f"xg{ch}", name=f"xg{ch}")
                    # k=0 rows (x, pre-permuted on host) + the ones row
                    nc.gpsimd.dma_start(xg_t[0:C, :], xg0[0:C, ch * 128 * B:(ch + 1) * 128 * B])
                    nc.gpsimd.dma_start(xg_t[KC:KC + 1, :], xg0[C:C + 1, ch * 128 * B:(ch + 1) * 128 * B])
                    xgs.append(xg_t)

                with tc.tile_pool(name="ptp", bufs=1) as ptp, \
                     tc.tile_pool(name="stream", bufs=3) as stream:
                    pt = ptp.tile([128, MT * NL], BF16, tag="pt")  # PT[m%128, mt*NL+n]

                    # ---- transposed exp-scores + row sums -----------------
                    with tc.tile_pool(name="etp", bufs=2) as etp, \
                         tc.tile_pool(name="ps_sc", bufs=4, space="PSUM") as ps_sc:
                        accs = [ptp.tile([128, NL], F32, tag=f"accs{i}", name=f"accs{i}")
                                for i in range(2)]
                        et_c = None
                        for mt in range(MT):
                            if mt % 8 == 0:
                                et_c = etp.tile([D, 1024], F32R, tag="etc")
                                nc.scalar.dma_start(
                                    et_c[:],
                                    et[:, mt * 128:(mt + 8) * 128].bitcast(F32R))
                            s_ps = ps_sc.tile([128, NL], F32, tag="s")
                            nc.tensor.matmul(
                                s_ps[:],
                                et_c[:, (mt % 8) * 128:(mt % 8 + 1) * 128],
                                etl_sb[:],
                                start=True, stop=True,
                            )
                            pslice = pt[:, mt * NL:(mt + 1) * NL]
                            nc.scalar.activation(pslice, s_ps[:], AF.Exp)
                            nc.vector.tensor_scalar_max(pslice, pslice, 1.0)
                            a = accs[mt % 2]
                            if mt < 2:
                                nc.vector.tensor_copy(a[:], pslice)
                            else:
                                nc.vector.tensor_tensor(
                                    a[:], a[:], pslice, mybir.AluOpType.add)
                        nc.vector.tensor_tensor(
                            accs[0][:], accs[0][:], accs[1][:], mybir.AluOpType.add)
                        acc_fin = accs[0]

                    # ---- hop1: two column sweeps, AG per sweep ------------
                    st1s = {}
                    with tc.tile_pool(name="ps_h1", bufs=1, space="PSUM") as ps_h1:
                        xt_hs = [xt_h0, xt_h1]
                        for h in range(2):
                            acc = [ps_h1.tile([128, H], F32, tag=f"acc{nt_i}",
                                              name=f"acc{h}_{nt_i}")
                                   for nt_i in range(NT)]
                            for k4 in range(MT // 4):
                                rt = stream.tile([128, 4 * H], BF16, tag="rhs", bufs=3)
                                nc.sync.dma_start(
                                    rt[:].rearrange("p (f c) -> p f c", f=4),
                                    xt_hs[h][k4 * 512:(k4 + 1) * 512, :]
                                    .rearrange("(f p) c -> p f c", p=128))
                                for f in range(4):
                                    k = k4 * 4 + f
                                    for nt_i in range(NT):
                                        lhs = pt[:, k * NL + nt_i * 128: k * NL + (nt_i + 1) * 128]
                                        nc.tensor.matmul(
                                            acc[nt_i][:], lhs, rt[:, f * H:(f + 1) * H],
                                            start=(k == 0), stop=(k == MT - 1),
                                        )
                            if h == 0:
                                # row sums -> 1/Z, 2/Z (overlaps sweep B)
                                with tc.tile_pool(name="ps_zs", bufs=1, space="PSUM") as ps_zs:
                                    zs = ps_zs.tile([128, 2 * NT], F32, tag="zs")
                                    for nt_i in range(NT):
                                        nc.tensor.matmul(
                                            zs[:, 2 * nt_i:2 * nt_i + 2],
                                            acc_fin[:, nt_i * 128:(nt_i + 1) * 128],
                                            ones_f[:],
                                            start=True, stop=True,
                                        )
                                    for nt_i in range(NT):
                                        nc.vector.reciprocal(
                                            r1[:, nt_i:nt_i + 1], zs[:, 2 * nt_i:2 * nt_i + 1])
                                        nc.vector.tensor_scalar_mul(
                                            r2[:, nt_i:nt_i + 1], r1[:, nt_i:nt_i + 1], 2.0)
                            # epilogue: scale by 1/Z, stage AG input, launch AG
                            for nt_i in range(NT):
                                st_bf = stream.tile([128, H], BF16, tag=f"z1_{h}_{nt_i}",
                                                    name=f"z1_{h}_{nt_i}", bufs=1)
                                nc.vector.tensor_scalar(
                                    st_bf[:], acc[nt_i][:], r1[:, nt_i:nt_i + 1], None,
                                    op0=mybir.AluOpType.mult,
                                )
                                nc.gpsimd.dma_start(
                                    ag_ins[h][nt_i * 128:(nt_i + 1) * 128, :], st_bf[:])
                                st1s[(h, nt_i)] = st_bf
                            nc.gpsimd.collective_compute(
                                "AllGather",
                                mybir.AluOpType.bypass,
                                ins=[ag_ins[h].opt()],
                                outs=[ag_outs[h].opt()],
                                replica_groups=[list(range(NC_CORES))],
                            )
                            # z1 -> scr1 (c-major scatter; feeds xg k=1 rows)
                            for nt_i in range(NT):
                                nc.scalar.dma_start(
                                    scr1[h * 16:(h + 1) * 16, nt_i * 128:(nt_i + 1) * 128, :]
                                    .transpose((1, 0, 2)),
                                    st1s[(h, nt_i)][:].rearrange("p (c b) -> p c b", b=B),
                                )
                            for ch in range(NT):
                                nc.gpsimd.dma_start(
                                    xgs[ch][C + 16 * h:C + 16 * (h + 1), :]
                                    .rearrange("c (n b) -> c n b", b=B),
                                    scr1[h * 16:(h + 1) * 16, ch * 128:(ch + 1) * 128, :],
                                )

                    # ---- weight-slab generation (4x row-tiled) ------------
                    with tc.tile_pool(name="ps_wt", bufs=4, space="PSUM") as ps_wt:
                        for o in range(O):
                            g, j = o % 3, o // 3
                            w_ps = ps_wt.tile([KC, H], F32, tag="wps")
                            nc.tensor.matmul(
                                w_ps[:],
                                wpo_rep[32 * g:32 * g + D, j * KC:(j + 1) * KC],
                                etl_rep[32 * g:32 * g + D, :],
                                start=True, stop=True,
                            )
                            src_v = w_ps[:].rearrange("p (nh nl) -> p nh nl", nl=8)
                            if o % 2 == 0:
                                nc.vector.tensor_copy(wt_i8[:, :, o, :], src_v)
                            else:
                                nc.scalar.activation(wt_i8[:, :, o, :], src_v, AF.Copy)

                    # ---- hop2: two column sweeps + pipelined grouped GEMM -
                    with tc.tile_pool(name="ps_h2", bufs=1, space="PSUM") as ps_h2, \
                         tc.tile_pool(name="ps_g", bufs=2, space="PSUM") as ps_g:
                        for h in range(2):
                            acc2 = [ps_h2.tile([128, H], F32, tag=f"a2_{nt_i}",
                                               name=f"a2_{h}_{nt_i}")
                                    for nt_i in range(NT)]
                            for k4 in range(MT // 4):
                                rt = stream.tile([128, 4 * H], BF16, tag="rhs", bufs=3)
                                nc.sync.dma_start(
                                    rt[:].rearrange("p (f c) -> p f c", f=4),
                                    ag_outs[h][k4 * 512:(k4 + 1) * 512, :]
                                    .rearrange("(f p) c -> p f c", p=128))
                                for f in range(4):
                                    k = k4 * 4 + f
                                    for nt_i in range(NT):
                                        lhs = pt[:, k * NL + nt_i * 128: k * NL + (nt_i + 1) * 128]
                                        nc.tensor.matmul(
                                            acc2[nt_i][:], lhs, rt[:, f * H:(f + 1) * H],
                                            start=(k == 0), stop=(k == MT - 1),
                                        )
                            # epilogue for this column half (h=0 overlaps h=1 matmuls)
                            for nt_i in range(NT):
                                st = stream.tile([128, H], F32, tag="zst", bufs=2)
                                nc.scalar.activation(
                                    st[:], acc2[nt_i][:], AF.Copy,
                                    scale=r2[:, nt_i:nt_i + 1],
                                )
                                st_bf = stream.tile([128, H], BF16, tag="zstb", bufs=2)
                                nc.vector.tensor_tensor(
                                    st_bf[:], st[:],
                                    xloc_sb[:, nt_i * BC + h * H: nt_i * BC + (h + 1) * H],
                                    mybir.AluOpType.subtract,
                                )
                                nc.gpsimd.dma_start(
                                    scr2[h * 16:(h + 1) * 16, nt_i * 128:(nt_i + 1) * 128, :]
                                    .transpose((1, 0, 2)),
                                    st_bf[:].rearrange("p (c b) -> p c b", b=B),
                                )
                            for ch in range(NT):
                                eng = nc.scalar if ch % 2 == 0 else nc.gpsimd
                                eng.dma_start(
                                    xgs[ch][2 * C + 16 * h:2 * C + 16 * (h + 1), :]
                                    .rearrange("c (n b) -> c n b", b=B),
                                    scr2[16 * h:16 * (h + 1), ch * 128:(ch + 1) * 128, :],
                                )

                        for ch in range(NT):
                            n0 = ch * 128
                            xg_b = xgs[ch]
                            for q16 in range(4):  # 32 nodes per psum tile
                                g_ps = ps_g.tile([128, 512], F32, tag="gps")
                                for j in range(8):
                                    for g in range(4):
                                        nl_i = q16 * 32 + j * 4 + g
                                        n_gl = n0 + nl_i
                                        nc.tensor.matmul(
                                            g_ps[32 * g:32 * (g + 1), j * O:(j + 1) * O],
                                            xg_b[:, nl_i * B:(nl_i + 1) * B],
                                            wt_g[:, n_gl // 8, n_gl % 8, :],
                                            start=True, stop=True,
                                            tile_position=(0, 32 * g),
                                        )
                                st = tstream.tile([128, 512], F32, tag="gst", bufs=2)
                                nc.vector.tensor_copy(st[:], g_ps[:])
                                dst4 = out_loc[n0 + q16 * 32: n0 + (q16 + 1) * 32, :, :] \
                                    .rearrange("(j g) b o -> g b j o", g=4)
                                for g in range(4):
                                    eng = (nc.sync, nc.gpsimd, nc.scalar, nc.sync)[g]
                                    eng.dma_start(
                                        dst4[g],
                                        st[32 * g:32 * (g + 1), :].rearrange(
                                            "b (j o) -> b j o", o=O),
                                    )
                xgp_cm.__exit__(None, None, None)
    return out_loc


_COMPILED = None


def _get_compiled():
    global _COMPILED
    if _COMPILED is None:
        nc = bacc.Bacc(
            "TRN2",
            target_bir_lowering=False,
            debug=False,
            num_devices=NC_CORES,
        )
        _build(nc)
        nc.compile()
        _COMPILED = nc
    return _COMPILED


def kernel(x, node_embeddings, laplacian_mx, weights_pool, bias_pool):
    x = np.asarray(x, dtype=np.float32)
    e = np.asarray(node_embeddings, dtype=np.float32)
    wp = np.asarray(weights_pool, dtype=np.float32)
    bp = np.asarray(bias_pool, dtype=np.float32)

    et = np.ascontiguousarray(e.T)                                    # [D, N]
    xt_h = np.ascontiguousarray(x.transpose(1, 2, 0).reshape(N, BC))  # [m, c*32+b]
    xt_b = xt_h.astype(ml_dtypes.bfloat16)
    xt_b0 = np.ascontiguousarray(xt_b[:, :H])
    xt_b1 = np.ascontiguousarray(xt_b[:, H:])
    bias_h = (e @ bp).astype(np.float32)                              # [N, O]

    # wpo4[g] = [d, (j, kc)] where o = j*3 + g (padded to 22 j-slots)
    wpo = wp.reshape(D, KC, O)                                        # [d, kc, o]
    wpo4 = np.zeros((3, D, 22, KC), dtype=np.float32)
    for g in range(3):
        sel = wpo[:, :, g::3]                                         # [d, kc, nj]
        wpo4[g, :, :sel.shape[2], :] = sel.transpose(0, 2, 1)
    wpo4 = np.ascontiguousarray(wpo4.reshape(3, D, 22 * KC)).astype(ml_dtypes.bfloat16)

    et_bf = et.astype(ml_dtypes.bfloat16)
    ones_row = np.ones((1, NL * B), dtype=ml_dtypes.bfloat16)

    in_maps = []
    for i in range(NC_CORES):
        sl = slice(i * NL, (i + 1) * NL)
        # xg k=0 slab: [c, n, b] for local nodes + ones row
        xg0_i = np.concatenate([
            np.ascontiguousarray(x[:, sl, :].transpose(2, 1, 0)).reshape(C, NL * B)
            .astype(ml_dtypes.bfloat16),
            ones_row,
        ], axis=0)
        in_maps.append({
            "et": et,
            "et_loc": np.ascontiguousarray(et[:, sl]),
            "et_loc_bf": np.ascontiguousarray(et_bf[:, sl]),
            "xt_h0": xt_b0,
            "xt_h1": xt_b1,
            "xt_loc": np.ascontiguousarray(xt_h[sl]),
            "xg0": xg0_i,
            "wpo4": wpo4,
            "bias_flat": np.ascontiguousarray(
                bias_h[sl].reshape(64, 8, O).transpose(0, 2, 1).reshape(1, NL * O)
                .astype(ml_dtypes.bfloat16)
            ),
        })

    nc = _get_compiled()
    trace = bool(int(os.environ.get("KBENCH_TRACE", "0")))
    if trace:
        trace = _register_ntff_hook()
    res = run_bass_kernel_spmd(
        nc,
        in_maps,
        core_ids=list(range(NC_CORES)),
        trace=trace,
    )
    LAST_RESULTS["exec_time_ns"] = res.exec_time_ns
    LAST_RESULTS["trace"] = res.instructions_and_trace
    LAST_RESULTS["mean_exec_time_ns"] = res.mean_exec_time_ns

    out = np.empty((B, N, O), dtype=np.float32)
    for i in range(NC_CORES):
        out[:, i * NL:(i + 1) * NL, :] = res.results[i]["out_loc"].transpose(1, 0, 2)
    return out
